# revision 1
# baseline (speedup 1.0000x reference)
import sys
sys.path.insert(0, '/opt/trn_rl_repo')
import numpy as np
import ml_dtypes
import concourse.bass as bass
import concourse.bacc as bacc
import concourse.tile as tile
import concourse.mybir as mybir
from concourse.bass_utils import run_bass_kernel_spmd

C3_TABLE = [(0, 1, 2), (1, 2, 3), (2, 3, 4), (3, 4, 5), (0, 4, 5), (0, 1, 5),
            (0, 1, 2, 3), (1, 2, 3, 4), (2, 3, 4, 5), (0, 3, 4, 5), (0, 1, 4, 5),
            (0, 1, 2, 5), (0, 1, 3, 4), (1, 2, 4, 5), (0, 2, 3, 5),
            (0, 1, 2, 3, 4, 5)]
A = 1.7159
S = 2.0 / 3.0
Q = 127.0                      # int8 quant scale for tanh in [-1, 1]

B, C, H, W = 256, 6, 142, 142
KH = KW = 5
OC = 16
OH, OW = H - 4, W - 4          # 138
NCORES = 8
B_LOC = B // NCORES            # 32
BF16 = ml_dtypes.bfloat16

T = 6                          # oh rows per block
HH = T + KH - 1                # 10
NS2 = 2                        # kw taps packed into K (s dim)
K = C * HH * NS2               # 120
M = OC * T                     # 96
NP = 3                         # matmul passes: kw pairs {0,1},{2,3},{4,-}
BPER = 2
NS = BPER * OW                 # 276
NBLK = OH // T                 # 23
NPAIR = B_LOC // BPER          # 16
NQ = 4                         # psum groups per block (round-robin 2 psum tiles)
QP = NPAIR // NQ               # 4 pairs per group
QI = QP * BPER                 # 8 batches per group
QF = QI * W                    # 1136 x cols per group
XF = B_LOC * W                 # 4544
PB = 512                       # psum bank stride in fp32 elems
QS = QP * NS                   # 1104 stage cols per group
SF = NQ * QS                   # 4416

_cache = {}


def _build():
    if 'nc' in _cache:
        return _cache['nc']
    f32 = mybir.dt.float32
    bf16 = mybir.dt.bfloat16
    i8 = mybir.dt.int8
    nc = bacc.Bacc("TRN2", target_bir_lowering=False, debug=False,
                   num_devices=NCORES)
    x_d = nc.dram_tensor("x", [NBLK, K, B_LOC, W], bf16, kind="ExternalInput").ap()
    w_d = nc.dram_tensor("w", [K, NP, M], bf16, kind="ExternalInput").ap()
    b_d = nc.dram_tensor("b", [M, 1], f32, kind="ExternalInput").ap()
    y_d = nc.dram_tensor("y", [NBLK, M, SF], i8, kind="ExternalOutput").ap()

    with tile.TileContext(nc) as tc:
        with tc.tile_pool(name="wpool", bufs=1) as wpool, \
             tc.tile_pool(name="xpool", bufs=6) as xpool, \
             tc.tile_pool(name="tpool", bufs=6) as tpool, \
             tc.tile_pool(name="spool", bufs=4) as spool, \
             tc.tile_pool(name="pspool", bufs=1, space="PSUM") as pspool:

            w_sb = wpool.tile([K, NP * M], bf16)
            b_sb = wpool.tile([M, 1], f32)

            for blk in range(NBLK):
                xt = xpool.tile([K, XF], bf16)
                if blk == 0:
                    # quarter-granular loads so the first matmul starts early;
                    # the small weight/bias tables transfer while q0 streams
                    for q in range(NQ):
                        src = x_d[blk, :, q * QI:(q + 1) * QI, :]
                        nc.sync.dma_start(xt[:, q * QF:(q + 1) * QF],
                                          src.rearrange("k i w -> k (i w)"))
                        if q == 0:
                            nc.sync.dma_start(
                                w_sb[:], w_d[:].rearrange("k f m -> k (f m)"))
                            nc.sync.dma_start(b_sb[:], b_d[:])
                else:
                    for hh_ in range(2):
                        src = x_d[blk, :, hh_ * 2 * QI:(hh_ + 1) * 2 * QI, :]
                        nc.sync.dma_start(
                            xt[:, hh_ * 2 * QF:(hh_ + 1) * 2 * QF],
                            src.rearrange("k i w -> k (i w)"))

                stage = spool.tile([M, SF], i8)
                xv = xt[:].rearrange("k (i w) -> k i w", i=B_LOC)
                pss = [pspool.tile([M, QP * PB], f32, name=f"ps{h_}",
                                   tag=f"ps{h_}") for h_ in range(2)]
                for q in range(NQ):
                    ps = pss[q % 2]
                    for g in range(NP):
                        for p in range(QP):
                            pair = q * QP + p
                            rv = xv[:, pair * BPER:(pair + 1) * BPER, :]
                            nc.tensor.matmul(
                                ps[:, p * PB:p * PB + NS],
                                w_sb[:, g * M:(g + 1) * M],
                                rv[:, :, 2 * g:2 * g + OW],
                                start=(g == 0), stop=(g == NP - 1),
                            )
                    src_v = ps[:].rearrange("m (p n) -> m p n", n=PB)[:, :, 0:NS]
                    if q % 2 == 0:
                        t_sb = tpool.tile([M, 2 * QS], bf16)
                    t_sl = t_sb[:, (q % 2) * QS:(q % 2 + 1) * QS]
                    t_v = t_sl.rearrange("m (p n) -> m p n", n=NS)
                    nc.scalar.activation(
                        t_v, src_v, mybir.ActivationFunctionType.Tanh,
                        bias=b_sb[:], scale=S)
                    if q % 2 == 1:
                        nc.vector.tensor_scalar_mul(
                            stage[:, (q - 1) * QS:(q + 1) * QS], t_sb[:], Q)
                if blk < NBLK - 1:
                    nc.gpsimd.dma_start(y_d[blk], stage[:])
                else:
                    # tail: per-half output so transfers overlap final drains
                    for hh_ in range(2):
                        nc.gpsimd.dma_start(
                            y_d[blk, :, hh_ * 2 * QS:(hh_ + 1) * 2 * QS],
                            stage[:, hh_ * 2 * QS:(hh_ + 1) * 2 * QS])
    nc.compile()
    _cache['nc'] = nc
    return nc


def _prep_weights(w3, b3, w4, b4, w6, b6):
    Wd = np.zeros((OC, C, KH, KW), np.float32)
    bias = np.zeros((OC,), np.float32)
    for i, idx in enumerate(C3_TABLE[:6]):
        Wd[i, list(idx)] = w3[i]
        bias[i] = b3[i]
    for i, idx in enumerate(C3_TABLE[6:15]):
        Wd[6 + i, list(idx)] = w4[i]
        bias[6 + i] = b4[i]
    Wd[15, list(C3_TABLE[15])] = w6[0]
    bias[15] = b6[0]

    # K row r = (c*HH + hh)*2 + s ; M col m = oc*T + j ; pass g: kw = 2g+s
    wk = np.zeros((K, NP, M), np.float32)
    for c in range(C):
        for hh in range(HH):
            for j in range(T):
                kh = hh - j
                if not (0 <= kh < KH):
                    continue
                for s in range(NS2):
                    for g in range(NP):
                        kw = 2 * g + s
                        if kw < KW:
                            r = (c * HH + hh) * 2 + s
                            wk[r, g, np.arange(OC) * T + j] = Wd[:, c, kh, kw]
    bvec = (S * bias[np.arange(M) // T]).reshape(M, 1).astype(np.float32)
    return wk.astype(BF16), bvec


def _prep_x(x_shard):
    # [B_LOC, C, H, W] -> [NBLK, K=(c,hh,s), B_LOC, W]; s=1 shifted by one col
    xt = x_shard.transpose(1, 2, 0, 3)                  # [C, H, B, W]
    xb = np.zeros((NBLK, C, HH, NS2, B_LOC, W), BF16)
    rows = (np.arange(NBLK) * T)[:, None] + np.arange(HH)[None, :]  # [23,10]
    g = xt[:, rows].astype(BF16)                        # [C, 23, 10, B, W]
    g = g.transpose(1, 0, 2, 3, 4)                      # [23, C, 10, B, W]
    xb[:, :, :, 0, :, :] = g
    xb[:, :, :, 1, :, :-1] = g[..., 1:]
    return np.ascontiguousarray(xb.reshape(NBLK, K, B_LOC, W))


def _unpack_y(y_s):
    v = np.asarray(y_s).astype(np.float32)
    v = v.reshape(NBLK, OC, T, NPAIR, BPER, OW)
    v = v.transpose(3, 4, 1, 0, 2, 5)                   # pair,b2,oc,blk,j,ow
    return v.reshape(B_LOC, OC, OH, OW)


def kernel(x, w3, b3, w4, b4, w6, b6):
    nc = _build()
    w3, b3, w4, b4, w6, b6 = [np.asarray(a, dtype=np.float32)
                              for a in (w3, b3, w4, b4, w6, b6)]
    wk, bvec = _prep_weights(w3, b3, w4, b4, w6, b6)
    x = np.ascontiguousarray(np.asarray(x), dtype=np.float32)
    in_maps = [{"x": _prep_x(x[i * B_LOC:(i + 1) * B_LOC]), "w": wk, "b": bvec}
               for i in range(NCORES)]
    res = run_bass_kernel_spmd(nc, in_maps, list(range(NCORES)))
    out = np.concatenate([_unpack_y(res.results[i]["y"]) for i in range(NCORES)],
                         axis=0)
    out *= (A / Q)
    return np.ascontiguousarray(out)

